# revision 4
# baseline (speedup 1.0000x reference)
"""Trainium2 Bass kernel for NEAT-style genome evaluation (gnn_message_passing).

Shapes are hardcoded for the problem:
  inputs [16384, 256] f32, in_idx/edge_w/edge_mask [768, 8], bias/response [768],
  out_idx [768] (scan order), output_idx [64]; output [16384, 64] f32.

Strategy: pure data-parallel over batch (2048 rows/core on 8 cores). The host
pre-transposes the input to node-major fp16 so DMA lands directly in the SBUF
value store v[chunk] = [128 nodes, 2048 batch] (no on-device marshaling).
Nodes are packed by topological level into 128-row chunks; per (level, pass)
the pre-activations accumulate in PSUM via fp16 matmuls whose [128,128]
stationary blocks embed the sparse DAG edges, and tanh(bias + response*s)
runs on the scalar engine over [128 rows, 1024 batch].  The batch is processed
in two sequential passes of 1024 columns so each chunk's PSUM tile is only
2 banks, allowing 4 live tiles and a 3-chunk cross-chunk matmul prefetch
horizon; cross-chunk ("parta") matmul quarters are greedily interleaved into
the level cascade as fillers so the tensor engine stays busy under the serial
activation chain.  Output node rows are DMA'd fp16 node-major; the host
gathers/transposes/converts.  fp16 keeps relative error ~3e-3 (tolerance 2e-2).
"""

import sys

import numpy as np

if "/opt/trn_rl_repo" not in sys.path:
    sys.path.insert(0, "/opt/trn_rl_repo")

import concourse.bacc as bacc
import concourse.mybir as mybir
from concourse.bass_utils import run_bass_kernel_spmd
from concourse.tile import TileContext

F16 = mybir.dt.float16
F32 = mybir.dt.float32

B = 16384
NUM_IN = 256
N = 1024
K = 8
NN = N - NUM_IN
NUM_OUT = 64
NCORES = 8
BC = B // NCORES          # batch rows per core (2048)
HALF = BC // 2            # batch columns per pass (1024)
NQ = 2                    # 512-wide matmul quarters per pass
HORIZON = 3               # psum prefetch horizon in chunks


def _plan(in_idx, edge_mask, edge_w, bias, response, out_idx, output_idx):
    """All host-side graph analysis; returns the constant tensors + schedule."""
    in_idx = np.asarray(in_idx)
    edge_mask = np.asarray(edge_mask).astype(bool)
    edge_w = np.asarray(edge_w).astype(np.float32)
    bias = np.asarray(bias).astype(np.float32)
    response = np.asarray(response).astype(np.float32)
    out_idx = np.asarray(out_idx)
    output_idx = np.asarray(output_idx)

    # scan write position of each node (reference writes out_idx[r] at step r)
    write_pos = np.full(N, -1, dtype=np.int64)
    for r in range(NN):
        write_pos[out_idx[r]] = r

    # valid edges: mask set AND source reads a value written before this step
    valid = np.zeros((NN, K), dtype=bool)
    for r in range(NN):
        for k in range(K):
            if not edge_mask[r, k]:
                continue
            s = int(in_idx[r, k])
            if s < NUM_IN or (0 <= write_pos[s] < r):
                valid[r, k] = True

    # prune nodes that do not reach any output
    needed = np.zeros(N, dtype=bool)
    needed[output_idx] = True
    for r in range(NN - 1, -1, -1):
        d = out_idx[r]
        if needed[d]:
            for k in range(K):
                if valid[r, k]:
                    needed[in_idx[r, k]] = True

    # topological levels over reachable non-input nodes (inputs = level 0)
    level = np.zeros(N, dtype=np.int64)
    for r in range(NN):
        d = out_idx[r]
        if not needed[d]:
            continue
        lmax = 0
        for k in range(K):
            if valid[r, k]:
                lmax = max(lmax, level[in_idx[r, k]] + 1)
        level[d] = lmax
    depth = int(level[needed].max()) if needed.any() else 0

    # split any level wider than 128 (keeps chunk packing valid)
    groups = []  # arrays of node ids, dependency order
    for l in range(1, depth + 1):
        nodes = [out_idx[r] for r in range(NN)
                 if needed[out_idx[r]] and level[out_idx[r]] == l]
        nodes = np.array(sorted(nodes, key=lambda d: write_pos[d]), dtype=np.int64)
        for i in range(0, len(nodes), 128):
            groups.append(nodes[i:i + 128])

    # pack whole groups into 128-row node chunks
    chunks = []   # list of list[(group_nodes, local_start)]
    fill = 128
    for g in groups:
        if fill + len(g) > 128:
            chunks.append([])
            fill = 0
        chunks[-1].append((g, fill))
        fill += len(g)

    n_in_chunks = NUM_IN // 128          # 2
    n_node_chunks = len(chunks)
    n_chunks = n_in_chunks + n_node_chunks

    chunk_of = np.full(N, -1, dtype=np.int64)
    row_of = np.full(N, -1, dtype=np.int64)
    for j in range(NUM_IN):
        chunk_of[j] = j // 128
        row_of[j] = j % 128
    for ci, levs in enumerate(chunks):
        for g, start in levs:
            for i, d in enumerate(g):
                chunk_of[d] = n_in_chunks + ci
                row_of[d] = start + i

    # per-node bias/response laid out per chunk
    bias_c = np.zeros((128, n_node_chunks), dtype=np.float32)
    resp_c = np.ones((128, n_node_chunks), dtype=np.float32)
    for r in range(NN):
        d = out_idx[r]
        if not needed[d]:
            continue
        bias_c[row_of[d], chunk_of[d] - n_in_chunks] = bias[r]
        resp_c[row_of[d], chunk_of[d] - n_in_chunks] = response[r]

    # weight blocks
    wa_blocks = {}   # (dst_chunk_rel, src_chunk_abs) -> [128,128]
    wb_blocks = {}   # (dst_chunk_rel, group_idx_in_chunk) -> [128,128]
    for r in range(NN):
        d = out_idx[r]
        if not needed[d]:
            continue
        dc = chunk_of[d] - n_in_chunks
        for k in range(K):
            if not valid[r, k]:
                continue
            s = int(in_idx[r, k])
            w = float(edge_w[r, k])
            sc = chunk_of[s]
            if sc == chunk_of[d]:
                gi = next(i for i, (g, st) in enumerate(chunks[dc])
                          if st <= row_of[d] < st + len(g))
                blk = wb_blocks.setdefault((dc, gi), np.zeros((128, 128), np.float32))
            else:
                blk = wa_blocks.setdefault((dc, sc), np.zeros((128, 128), np.float32))
            blk[row_of[s], row_of[d]] += w

    parta = []   # per node chunk: list of (src_chunk, wa_index), src ascending
    wa_list = []
    for dc in range(n_node_chunks):
        lst = []
        for sc in range(n_chunks):
            if (dc, sc) in wa_blocks:
                lst.append((sc, len(wa_list)))
                wa_list.append(wa_blocks[(dc, sc)])
        parta.append(lst)

    partb = []   # per node chunk: list of (group_idx, local_start, m, wb_index|None)
    wb_list = []
    for dc in range(n_node_chunks):
        lst = []
        for gi, (g, st) in enumerate(chunks[dc]):
            if (dc, gi) in wb_blocks:
                lst.append((gi, st, len(g), len(wb_list)))
                wb_list.append(wb_blocks[(dc, gi)])
            else:
                lst.append((gi, st, len(g), None))
        partb.append(lst)

    # output extraction: row-ranges per chunk covering its output nodes
    rows_by_chunk = {}
    for d in output_idx:
        dc = int(chunk_of[d]) - n_in_chunks
        rows_by_chunk.setdefault(dc, []).append(int(row_of[d]))
    out_ranges = []   # (dst_chunk_rel, row0, len, col0)
    col = 0
    pos_of = {}       # (dc, row) -> staged row in o16
    last_oc = max(rows_by_chunk)
    for dc in sorted(rows_by_chunk):
        rows = sorted(set(rows_by_chunk[dc]))
        gap = 10**9 if dc == last_oc else 8
        start = prev = rows[0]
        for r in rows[1:] + [None]:
            if r is not None and r - prev <= gap:
                prev = r
                continue
            ln = prev - start + 1
            out_ranges.append((dc, start, ln, col))
            for rr in range(start, prev + 1):
                pos_of[(dc, rr)] = col + rr - start
            col += ln
            if r is not None:
                start = prev = r
    ntot = col
    assert ntot <= 192, f"staged output rows {ntot} too large"
    colmap = np.zeros(NUM_OUT, dtype=np.int64)
    for oc, d in enumerate(output_idx):
        dc = int(chunk_of[d]) - n_in_chunks
        colmap[oc] = pos_of[(dc, int(row_of[d]))]

    # host-packed weight images: [128, n*128] partition-major so one DMA row
    # per partition is contiguous in DRAM
    def pack(blocks):
        if not blocks:
            return np.zeros((128, 128), np.float16)
        arr = np.stack(blocks).astype(np.float16)        # [n, 128, 128]
        return np.ascontiguousarray(arr.transpose(1, 0, 2).reshape(128, -1))

    return dict(
        out_ranges=out_ranges,
        ntot=ntot,
        colmap=colmap,
        n_in_chunks=n_in_chunks,
        n_node_chunks=n_node_chunks,
        n_chunks=n_chunks,
        parta=parta,
        partb=partb,
        n_wa=len(wa_list),
        n_wb=len(wb_list),
        wa_pk=pack(wa_list),
        wb_pk=pack(wb_list),
        bias_c=bias_c,
        resp_c=resp_c,
    )


def _build_nc(plan):
    n_in_chunks = plan["n_in_chunks"]
    n_node_chunks = plan["n_node_chunks"]
    n_chunks = plan["n_chunks"]
    parta = plan["parta"]
    partb = plan["partb"]
    out_ranges = plan["out_ranges"]
    ntot = plan["ntot"]
    n_wa = max(plan["n_wa"], 1)
    n_wb = max(plan["n_wb"], 1)

    nc = bacc.Bacc()
    x = nc.dram_tensor("x", [NUM_IN, BC], F16, kind="ExternalInput")
    wa = nc.dram_tensor("wa", [128, n_wa * 128], F16, kind="ExternalInput")
    wb = nc.dram_tensor("wb", [128, n_wb * 128], F16, kind="ExternalInput")
    br_d = nc.dram_tensor("br_c", [128, 2 * n_node_chunks], F32, kind="ExternalInput")
    o = nc.dram_tensor("o", [ntot, BC], F16, kind="ExternalOutput")

    with TileContext(nc) as tc:
        with tc.tile_pool(name="const", bufs=1) as const, \
             tc.tile_pool(name="vpool", bufs=n_chunks) as vpool, \
             tc.tile_pool(name="pc", bufs=4, space="PSUM") as pcp:

            br_sb = const.tile([128, 2 * n_node_chunks], F32, tag="br")
            bias_sb = br_sb[:, 0:n_node_chunks]
            resp_sb = br_sb[:, n_node_chunks:2 * n_node_chunks]
            wa_sb = const.tile([128, n_wa * 128], F16, tag="wa_sb")
            wb_sb = const.tile([128, n_wb * 128], F16, tag="wb_sb")
            zt = const.tile([1, 2], F32, tag="zt")

            # node-major value store: v[chunk] = [128, 2048] fp16 (both passes)
            v = [vpool.tile([128, BC], F16, tag="v", name=f"v{c}")
             for c in range(n_chunks)]

            # trigger the tanh table load at t~0 on the scalar engine
            nc.vector.memset(zt[:], 0.0)
            nc.scalar.activation(zt[0:1, 1:2], zt[0:1, 0:1],
                                 mybir.ActivationFunctionType.Tanh)

            # ---- DMA schedule, urgency-ordered.  Pass-0 input quarters
            # first, then weights for the early chunks, then pass-1 input,
            # then the remaining weights.
            def dma_x(cin, q):
                nc.sync.dma_start(
                    v[cin][:, q * 512:(q + 1) * 512],
                    x[cin * 128:(cin + 1) * 128, q * 512:(q + 1) * 512])

            def wa_span(dc):
                idxs = [ai for _, ai in parta[dc]]
                return (idxs[0], idxs[-1] + 1) if idxs else None

            def wb_span(dc):
                bidx = [bi for _, _, _, bi in partb[dc] if bi is not None]
                return (bidx[0], bidx[-1] + 1) if bidx else None

            def dma_wa(i0, i1):
                nc.sync.dma_start(wa_sb[:, i0 * 128:i1 * 128],
                                  wa[:, i0 * 128:i1 * 128])

            def dma_wb(i0, i1):
                nc.sync.dma_start(wb_sb[:, i0 * 128:i1 * 128],
                                  wb[:, i0 * 128:i1 * 128])

            for cin in range(n_in_chunks):
                for q in range(NQ):
                    dma_x(cin, q)
            nc.scalar.dma_start(br_sb[:], br_d[:])
            s = wa_span(0)
            if s:
                dma_wa(*s)
            s = wb_span(0)
            if s:
                dma_wb(*s)
            s0 = wa_span(1)
            s1 = wb_span(1)
            if s0:
                dma_wa(*s0)
            if s1:
                dma_wb(*s1)
            # pass-1 input columns
            for cin in range(n_in_chunks):
                for q in range(NQ):
                    dma_x(cin, NQ + q)
            # remaining weights in two pieces each
            a_lo = wa_span(2)[0] if n_node_chunks > 2 and wa_span(2) else plan["n_wa"]
            if a_lo < plan["n_wa"]:
                mid = (a_lo + plan["n_wa"] + 1) // 2
                dma_wa(a_lo, mid)
                dma_wa(mid, plan["n_wa"])
            b_lo = wb_span(2)[0] if n_node_chunks > 2 and wb_span(2) else plan["n_wb"]
            if b_lo < plan["n_wb"]:
                dma_wb(b_lo, plan["n_wb"])

            # ---- cascade with the two batch passes interleaved at chunk
            # granularity: A0 A1 B0 A2 B1 ... B4 A6 B5 B6.  Chunk
            # transitions then land between independent streams, so the
            # previous chunk's cross-chunk matmuls have a full chunk window
            # to complete and the activation chain never bubbles.
            order = [(0, 0), (0, 1)]
            for dc in range(2, n_node_chunks):
                order.append((1, dc - 2))
                order.append((0, dc))
            order.append((1, n_node_chunks - 2))
            order.append((1, n_node_chunks - 1))
            assert len(order) == 2 * n_node_chunks
            gpos = {pc_: g for g, pc_ in enumerate(order)}   # (P, dc) -> g
            n_g = len(order)

            pc_tiles = [None] * n_g        # psum tile per global chunk
            started = [[False] * NQ for _ in range(n_g)]   # per quarter

            # filler units: one 512-wide matmul each.  ready = global step
            # at which the source values exist; deadline = dst global step.
            units = []                     # (ready_g, dst_g, P, sc, ai, q)
            for g, (P, dc) in enumerate(order):
                for sc, ai in parta[dc]:
                    if sc < n_in_chunks:
                        ready = 0
                    else:
                        ready = gpos[(P, sc - n_in_chunks)] + 1
                    for q in range(NQ):
                        units.append((ready, g, P, sc, ai, q))
            units.sort(key=lambda u: (u[1], u[5], u[3]))
            emitted = [False] * len(units)

            def get_pc(g):
                if pc_tiles[g] is None:
                    pc_tiles[g] = pcp.tile([128, HALF], F32, tag="pc",
                                           name=f"pc{g}")
                return pc_tiles[g]

            def emit_unit(i):
                ready, g, P, sc, ai, q = units[i]
                pc = get_pc(g)
                nc.tensor.matmul(
                    pc[:, q * 512:(q + 1) * 512],
                    wa_sb[:, ai * 128:(ai + 1) * 128],
                    v[sc][:, P * HALF + q * 512:P * HALF + (q + 1) * 512],
                    start=not started[g][q], stop=False,
                    skip_group_check=True)
                started[g][q] = True
                emitted[i] = True

            def emit_fillers(cur_g, budget):
                n = 0
                for i, u in enumerate(units):
                    if n >= budget:
                        break
                    if emitted[i] or u[0] > cur_g or u[1] > cur_g + HORIZON:
                        continue
                    emit_unit(i)
                    n += 1

            for g, (P, dc) in enumerate(order):
                gc = n_in_chunks + dc
                # this chunk's own remaining parta must be in before its acts
                for i, u in enumerate(units):
                    if not emitted[i] and u[1] == g:
                        emit_unit(i)
                wbts = {gi: wb_sb[:, bi * 128:(bi + 1) * 128]
                        for gi, st, m, bi in partb[dc] if bi is not None}
                pc = get_pc(g)
                for idx, (gi, st, m, bi) in enumerate(partb[dc]):
                    if bi is not None:
                        for q in range(NQ):
                            nc.tensor.matmul(
                                pc[:, q * 512:(q + 1) * 512],
                                wbts[gi],
                                v[gc][:, P * HALF + q * 512:P * HALF + (q + 1) * 512],
                                start=False, stop=False,
                                skip_group_check=True)
                        budget = 2
                    else:
                        budget = 4
                    # full-chunk tanh: earlier levels recompute identical
                    # values, later rows get garbage that is overwritten
                    # before any true read (wb stationaries are zero there)
                    nc.scalar.activation(
                        v[gc][:, P * HALF:(P + 1) * HALF], pc[:, :],
                        mybir.ActivationFunctionType.Tanh,
                        bias=bias_sb[:, dc:dc + 1],
                        scale=resp_sb[:, dc:dc + 1])
                    emit_fillers(g, budget)
                # after the final pass of a chunk, stream its output rows out
                if P == 1:
                    rgs = [(r0, ln, c0) for c, r0, ln, c0 in out_ranges if c == dc]
                    for r0, ln, c0 in rgs:
                        nc.sync.dma_start(o[c0:c0 + ln, :],
                                          v[gc][r0:r0 + ln, :])

    nc.compile()
    return nc


_CACHE = {}


def _get_compiled(key, plan):
    if key not in _CACHE:
        _CACHE[key] = _build_nc(plan)
    return _CACHE[key]


def kernel(inputs, edge_w, bias, response, in_idx, edge_mask, out_idx, output_idx):
    inputs = np.ascontiguousarray(np.asarray(inputs, dtype=np.float32))
    plan = _plan(in_idx, edge_mask, edge_w, bias, response, out_idx, output_idx)

    key = (plan["wa_pk"].tobytes(), plan["wb_pk"].tobytes(),
           str(plan["out_ranges"]), plan["bias_c"].tobytes(),
           plan["resp_c"].tobytes())
    nc = _get_compiled(hash(key), plan)

    base = {
        "wa": plan["wa_pk"],
        "wb": plan["wb_pk"],
        "br_c": np.ascontiguousarray(
            np.concatenate([plan["bias_c"], plan["resp_c"]], axis=1)),
    }

    x16 = inputs.astype(np.float16)
    in_maps = []
    for c in range(NCORES):
        m = dict(base)
        # node-major transpose: [256 nodes, 2048 batch]
        m["x"] = np.ascontiguousarray(x16[c * BC:(c + 1) * BC].T)
        in_maps.append(m)

    res = run_bass_kernel_spmd(nc, in_maps, core_ids=list(range(NCORES)))
    kernel.last_results = res
    colmap = np.asarray(plan["colmap"])
    out = np.concatenate(
        [res.results[c]["o"][colmap].T for c in range(NCORES)], axis=0)
    return np.ascontiguousarray(out.astype(np.float32))


kernel.last_results = None


# revision 7
# speedup vs baseline: 1.0984x; 1.0984x over previous
"""Trainium2 Bass kernel for NEAT-style genome evaluation (gnn_message_passing).

Shapes are hardcoded for the problem:
  inputs [16384, 256] f32, in_idx/edge_w/edge_mask [768, 8], bias/response [768],
  out_idx [768] (scan order), output_idx [64]; output [16384, 64] f32.

Strategy: pure data-parallel over batch (2048 rows/core on 8 cores). The host
pre-transposes the input to node-major fp16 so DMA lands directly in the SBUF
value store v[chunk] = [128 nodes, 2048 batch] (no on-device marshaling).
Nodes are packed by topological level into 128-row chunks; per (level, pass)
the pre-activations accumulate in PSUM via fp16 matmuls whose [128,128]
stationary blocks embed the sparse DAG edges, and tanh(bias + response*s)
runs on the scalar engine over [128 rows, 1024 batch].  The batch is processed
in two sequential passes of 1024 columns so each chunk's PSUM tile is only
2 banks, allowing 4 live tiles and a 3-chunk cross-chunk matmul prefetch
horizon; cross-chunk ("parta") matmul quarters are greedily interleaved into
the level cascade as fillers so the tensor engine stays busy under the serial
activation chain.  Output node rows are DMA'd fp16 node-major; the host
gathers/transposes/converts.  fp16 keeps relative error ~3e-3 (tolerance 2e-2).
"""

import sys

import numpy as np

if "/opt/trn_rl_repo" not in sys.path:
    sys.path.insert(0, "/opt/trn_rl_repo")

import concourse.bacc as bacc
import concourse.mybir as mybir
from concourse.bass_utils import run_bass_kernel_spmd
from concourse.tile import TileContext

F16 = mybir.dt.float16
F32 = mybir.dt.float32

B = 16384
NUM_IN = 256
N = 1024
K = 8
NN = N - NUM_IN
NUM_OUT = 64
NCORES = 8
BC = B // NCORES          # batch rows per core (2048)
HALF = BC // 2            # batch columns per pass (1024)
NQ = 2                    # 512-wide matmul quarters per pass
HORIZON = 3               # psum prefetch horizon in chunks


def _plan(in_idx, edge_mask, edge_w, bias, response, out_idx, output_idx):
    """All host-side graph analysis; returns the constant tensors + schedule."""
    in_idx = np.asarray(in_idx)
    edge_mask = np.asarray(edge_mask).astype(bool)
    edge_w = np.asarray(edge_w).astype(np.float32)
    bias = np.asarray(bias).astype(np.float32)
    response = np.asarray(response).astype(np.float32)
    out_idx = np.asarray(out_idx)
    output_idx = np.asarray(output_idx)

    # scan write position of each node (reference writes out_idx[r] at step r)
    write_pos = np.full(N, -1, dtype=np.int64)
    for r in range(NN):
        write_pos[out_idx[r]] = r

    # valid edges: mask set AND source reads a value written before this step
    valid = np.zeros((NN, K), dtype=bool)
    for r in range(NN):
        for k in range(K):
            if not edge_mask[r, k]:
                continue
            s = int(in_idx[r, k])
            if s < NUM_IN or (0 <= write_pos[s] < r):
                valid[r, k] = True

    # prune nodes that do not reach any output
    needed = np.zeros(N, dtype=bool)
    needed[output_idx] = True
    for r in range(NN - 1, -1, -1):
        d = out_idx[r]
        if needed[d]:
            for k in range(K):
                if valid[r, k]:
                    needed[in_idx[r, k]] = True

    # topological levels over reachable non-input nodes (inputs = level 0)
    level = np.zeros(N, dtype=np.int64)
    for r in range(NN):
        d = out_idx[r]
        if not needed[d]:
            continue
        lmax = 0
        for k in range(K):
            if valid[r, k]:
                lmax = max(lmax, level[in_idx[r, k]] + 1)
        level[d] = lmax
    depth = int(level[needed].max()) if needed.any() else 0

    # split any level wider than 128 (keeps chunk packing valid)
    groups = []  # arrays of node ids, dependency order
    for l in range(1, depth + 1):
        nodes = [out_idx[r] for r in range(NN)
                 if needed[out_idx[r]] and level[out_idx[r]] == l]
        nodes = np.array(sorted(nodes, key=lambda d: write_pos[d]), dtype=np.int64)
        for i in range(0, len(nodes), 128):
            groups.append(nodes[i:i + 128])

    # pack whole groups into 128-row node chunks
    chunks = []   # list of list[(group_nodes, local_start)]
    fill = 128
    for g in groups:
        if fill + len(g) > 128:
            chunks.append([])
            fill = 0
        chunks[-1].append((g, fill))
        fill += len(g)

    n_in_chunks = NUM_IN // 128          # 2
    n_node_chunks = len(chunks)
    n_chunks = n_in_chunks + n_node_chunks

    chunk_of = np.full(N, -1, dtype=np.int64)
    row_of = np.full(N, -1, dtype=np.int64)
    for j in range(NUM_IN):
        chunk_of[j] = j // 128
        row_of[j] = j % 128
    for ci, levs in enumerate(chunks):
        for g, start in levs:
            for i, d in enumerate(g):
                chunk_of[d] = n_in_chunks + ci
                row_of[d] = start + i

    # per-node bias/response laid out per chunk
    bias_c = np.zeros((128, n_node_chunks), dtype=np.float32)
    resp_c = np.ones((128, n_node_chunks), dtype=np.float32)
    for r in range(NN):
        d = out_idx[r]
        if not needed[d]:
            continue
        bias_c[row_of[d], chunk_of[d] - n_in_chunks] = bias[r]
        resp_c[row_of[d], chunk_of[d] - n_in_chunks] = response[r]

    # weight blocks
    wa_blocks = {}   # (dst_chunk_rel, src_chunk_abs) -> [128,128]
    wb_blocks = {}   # (dst_chunk_rel, group_idx_in_chunk) -> [128,128]
    for r in range(NN):
        d = out_idx[r]
        if not needed[d]:
            continue
        dc = chunk_of[d] - n_in_chunks
        for k in range(K):
            if not valid[r, k]:
                continue
            s = int(in_idx[r, k])
            w = float(edge_w[r, k])
            sc = chunk_of[s]
            if sc == chunk_of[d]:
                gi = next(i for i, (g, st) in enumerate(chunks[dc])
                          if st <= row_of[d] < st + len(g))
                blk = wb_blocks.setdefault((dc, gi), np.zeros((128, 128), np.float32))
            else:
                blk = wa_blocks.setdefault((dc, sc), np.zeros((128, 128), np.float32))
            blk[row_of[s], row_of[d]] += w

    parta = []   # per node chunk: list of (src_chunk, wa_index), src ascending
    wa_list = []
    for dc in range(n_node_chunks):
        lst = []
        for sc in range(n_chunks):
            if (dc, sc) in wa_blocks:
                lst.append((sc, len(wa_list)))
                wa_list.append(wa_blocks[(dc, sc)])
        parta.append(lst)

    partb = []   # per node chunk: list of (group_idx, local_start, m, wb_index|None)
    wb_list = []
    for dc in range(n_node_chunks):
        lst = []
        for gi, (g, st) in enumerate(chunks[dc]):
            if (dc, gi) in wb_blocks:
                lst.append((gi, st, len(g), len(wb_list)))
                wb_list.append(wb_blocks[(dc, gi)])
            else:
                lst.append((gi, st, len(g), None))
        partb.append(lst)

    # output extraction: row-ranges per chunk covering its output nodes
    rows_by_chunk = {}
    for d in output_idx:
        dc = int(chunk_of[d]) - n_in_chunks
        rows_by_chunk.setdefault(dc, []).append(int(row_of[d]))
    out_ranges = []   # (dst_chunk_rel, row0, len, col0)
    col = 0
    pos_of = {}       # (dc, row) -> staged row in o16
    last_oc = max(rows_by_chunk)
    for dc in sorted(rows_by_chunk):
        rows = sorted(set(rows_by_chunk[dc]))
        gap = 10**9 if dc == last_oc else 8
        start = prev = rows[0]
        for r in rows[1:] + [None]:
            if r is not None and r - prev <= gap:
                prev = r
                continue
            ln = prev - start + 1
            out_ranges.append((dc, start, ln, col))
            for rr in range(start, prev + 1):
                pos_of[(dc, rr)] = col + rr - start
            col += ln
            if r is not None:
                start = prev = r
    ntot = col
    assert ntot <= 192, f"staged output rows {ntot} too large"
    colmap = np.zeros(NUM_OUT, dtype=np.int64)
    for oc, d in enumerate(output_idx):
        dc = int(chunk_of[d]) - n_in_chunks
        colmap[oc] = pos_of[(dc, int(row_of[d]))]

    # host-packed weight images: [128, n*128] partition-major so one DMA row
    # per partition is contiguous in DRAM
    def pack(blocks):
        if not blocks:
            return np.zeros((128, 128), np.float16)
        arr = np.stack(blocks).astype(np.float16)        # [n, 128, 128]
        return np.ascontiguousarray(arr.transpose(1, 0, 2).reshape(128, -1))

    return dict(
        out_ranges=out_ranges,
        ntot=ntot,
        colmap=colmap,
        n_in_chunks=n_in_chunks,
        n_node_chunks=n_node_chunks,
        n_chunks=n_chunks,
        parta=parta,
        partb=partb,
        n_wa=len(wa_list),
        n_wb=len(wb_list),
        wa_pk=pack(wa_list),
        wb_pk=pack(wb_list),
        bias_c=bias_c,
        resp_c=resp_c,
    )


def _build_nc(plan):
    n_in_chunks = plan["n_in_chunks"]
    n_node_chunks = plan["n_node_chunks"]
    n_chunks = plan["n_chunks"]
    parta = plan["parta"]
    partb = plan["partb"]
    out_ranges = plan["out_ranges"]
    ntot = plan["ntot"]
    n_wa = max(plan["n_wa"], 1)
    n_wb = max(plan["n_wb"], 1)

    nc = bacc.Bacc()
    x = nc.dram_tensor("x", [NUM_IN, BC], F16, kind="ExternalInput")
    wa = nc.dram_tensor("wa", [128, n_wa * 128], F16, kind="ExternalInput")
    wb = nc.dram_tensor("wb", [128, n_wb * 128], F16, kind="ExternalInput")
    br_d = nc.dram_tensor("br_c", [128, 2 * n_node_chunks], F32, kind="ExternalInput")
    o = nc.dram_tensor("o", [ntot, BC], F16, kind="ExternalOutput")

    with TileContext(nc) as tc:
        with tc.tile_pool(name="const", bufs=1) as const, \
             tc.tile_pool(name="vpool", bufs=n_chunks) as vpool, \
             tc.tile_pool(name="pc", bufs=4, space="PSUM") as pcp:

            br_sb = const.tile([128, 2 * n_node_chunks], F32, tag="br")
            bias_sb = br_sb[:, 0:n_node_chunks]
            resp_sb = br_sb[:, n_node_chunks:2 * n_node_chunks]
            wa_sb = const.tile([128, n_wa * 128], F16, tag="wa_sb")
            wb_sb = const.tile([128, n_wb * 128], F16, tag="wb_sb")
            zt = const.tile([1, 2], F32, tag="zt")
            wz = const.tile([128, 512], F16, tag="wz")

            # node-major value store: v[chunk] = [128, 2048] fp16 (both passes)
            v = [vpool.tile([128, BC], F16, tag="v", name=f"v{c}")
             for c in range(n_chunks)]

            # trigger the tanh table load at t~0 on the scalar engine
            nc.vector.memset(zt[:], 0.0)
            nc.scalar.activation(zt[0:1, 1:2], zt[0:1, 0:1],
                                 mybir.ActivationFunctionType.Tanh)
            nc.vector.memset(wz[:], 0.0)

            # ---- DMA schedule, urgency-ordered: chunk-0 weights first, then
            # pass-A input (one DMA per input chunk), bias, chunk-1 weights,
            # pass-B input, remaining weights.
            def dma_x(cin, half):
                nc.sync.dma_start(
                    v[cin][:, half * HALF:(half + 1) * HALF],
                    x[cin * 128:(cin + 1) * 128, half * HALF:(half + 1) * HALF])

            def wa_span(dc):
                idxs = [ai for _, ai in parta[dc]]
                return (idxs[0], idxs[-1] + 1) if idxs else None

            def wb_span(dc):
                bidx = [bi for _, _, _, bi in partb[dc] if bi is not None]
                return (bidx[0], bidx[-1] + 1) if bidx else None

            def dma_wa(i0, i1):
                nc.sync.dma_start(wa_sb[:, i0 * 128:i1 * 128],
                                  wa[:, i0 * 128:i1 * 128])

            def dma_wb(i0, i1):
                nc.sync.dma_start(wb_sb[:, i0 * 128:i1 * 128],
                                  wb[:, i0 * 128:i1 * 128])

            s = wa_span(0)
            if s:
                dma_wa(*s)
            for cin in range(n_in_chunks):
                dma_x(cin, 0)
            nc.scalar.dma_start(br_sb[:], br_d[:])
            s = wb_span(0)
            if s:
                dma_wb(*s)
            for cin in range(n_in_chunks):
                dma_x(cin, 1)
            s0 = wa_span(1)
            s1 = wb_span(1)
            if s0:
                dma_wa(*s0)
            if s1:
                dma_wb(*s1)
            # remaining weights in pieces
            a_lo = wa_span(2)[0] if n_node_chunks > 2 and wa_span(2) else plan["n_wa"]
            if a_lo < plan["n_wa"]:
                mid = (a_lo + plan["n_wa"] + 1) // 2
                dma_wa(a_lo, mid)
                dma_wa(mid, plan["n_wa"])
            b_lo = wb_span(2)[0] if n_node_chunks > 2 and wb_span(2) else plan["n_wb"]
            if b_lo < plan["n_wb"]:
                dma_wb(b_lo, plan["n_wb"])

            # ---- PE warm-up: zero matmuls ramp the tensor-engine p-state
            # during the input DMA so the first real matmuls run full speed
            pc_tiles = {}                  # (P, dc) -> psum tile

            def get_pc(P, dc):
                if (P, dc) not in pc_tiles:
                    pc_tiles[(P, dc)] = pcp.tile([128, HALF], F32, tag="pc",
                                                 name=f"pc{P}_{dc}")
                return pc_tiles[(P, dc)]

            warm_pc = get_pc(0, 0)
            for _ in range(6):
                nc.tensor.matmul(warm_pc[:, 0:512], wz[:, 0:128], wz[:, :],
                                 start=True, stop=False, skip_group_check=True)

            # ---- cascade: the two batch passes (streams A=0, B=1) are
            # zipped at LEVEL granularity over the same chunk, B lagging one
            # level: A_L0, A_L1, B_L0, A_L2, B_L1, ..., B_L(last).  The act
            # queue is in-order, so alternating independent streams hides
            # each act's sem+matmul dependency path (~780ns) under the other
            # stream's 1038ns activation, keeping the scalar engine at full
            # cadence even when dependency-bound.
            started = {}                   # (P, dc, q) -> bool

            # filler units: one 512-wide cross-chunk matmul each.
            # ready_c = chunk pair after which the source values exist.
            units = []                     # [ready_c, dst_c, P, sc, ai, q]
            for dc in range(n_node_chunks):
                for P in range(2):
                    for sc, ai in parta[dc]:
                        ready = 0 if sc < n_in_chunks else sc - n_in_chunks + 1
                        for q in range(NQ):
                            units.append((ready, dc, P, sc, ai, q))
            units.sort(key=lambda u: (u[1], u[2], u[5], u[3]))
            emitted = [False] * len(units)
            acts_done = [[False] * n_node_chunks for _ in range(2)]

            def emit_unit(i):
                ready, dcu, P, sc, ai, q = units[i]
                pcu = get_pc(P, dcu)
                nc.tensor.matmul(
                    pcu[:, q * 512:(q + 1) * 512],
                    wa_sb[:, ai * 128:(ai + 1) * 128],
                    v[sc][:, P * HALF + q * 512:P * HALF + (q + 1) * 512],
                    start=not started.get((P, dcu, q), False), stop=False,
                    skip_group_check=True)
                started[(P, dcu, q)] = True
                emitted[i] = True

            def src_ready(i):
                r = units[i][0]
                return r == 0 or acts_done[units[i][2]][r - 1]

            def emit_fillers(cur_c, budget):
                n = 0
                for i in range(len(units)):
                    if n >= budget:
                        break
                    if emitted[i] or units[i][1] > cur_c + 1 or not src_ready(i):
                        continue
                    emit_unit(i)
                    n += 1

            def emit_act(P, dc, gi_idx):
                gc = n_in_chunks + dc
                gi, st, m, bi = partb[dc][gi_idx]
                pcu = get_pc(P, dc)
                if gi_idx == 0:
                    # all cross-chunk contributions must be in
                    for i in range(len(units)):
                        if not emitted[i] and units[i][1] == dc and units[i][2] == P:
                            emit_unit(i)
                if bi is not None:
                    wt = wb_sb[:, bi * 128:(bi + 1) * 128]
                    for q in range(NQ):
                        nc.tensor.matmul(
                            pcu[:, q * 512:(q + 1) * 512],
                            wt,
                            v[gc][:, P * HALF + q * 512:P * HALF + (q + 1) * 512],
                            start=False, stop=False,
                            skip_group_check=True)
                # full-chunk tanh: earlier levels recompute identical values,
                # later rows get garbage that is overwritten before any true
                # read (wb stationaries are zero there)
                nc.scalar.activation(
                    v[gc][:, P * HALF:(P + 1) * HALF], pcu[:, :],
                    mybir.ActivationFunctionType.Tanh,
                    bias=bias_sb[:, dc:dc + 1],
                    scale=resp_sb[:, dc:dc + 1])
                emit_fillers(dc, 2 if bi is not None else 4)
                if gi_idx == len(partb[dc]) - 1:
                    acts_done[P][dc] = True

            for dc in range(n_node_chunks):
                L = len(partb[dc])
                slots = [(0, 0)]
                for i in range(1, L):
                    slots.append((0, i))
                    slots.append((1, i - 1))
                slots.append((1, L - 1))
                for P, gi_idx in slots:
                    emit_act(P, dc, gi_idx)
                # stream this chunk's output rows out (both passes complete)
                rgs = [(r0, ln, c0) for c, r0, ln, c0 in out_ranges if c == dc]
                for r0, ln, c0 in rgs:
                    nc.sync.dma_start(o[c0:c0 + ln, :],
                                      v[n_in_chunks + dc][r0:r0 + ln, :])

    nc.compile()
    return nc


_CACHE = {}


def _get_compiled(key, plan):
    if key not in _CACHE:
        _CACHE[key] = _build_nc(plan)
    return _CACHE[key]


def kernel(inputs, edge_w, bias, response, in_idx, edge_mask, out_idx, output_idx):
    inputs = np.ascontiguousarray(np.asarray(inputs, dtype=np.float32))
    plan = _plan(in_idx, edge_mask, edge_w, bias, response, out_idx, output_idx)

    key = (plan["wa_pk"].tobytes(), plan["wb_pk"].tobytes(),
           str(plan["out_ranges"]), plan["bias_c"].tobytes(),
           plan["resp_c"].tobytes())
    nc = _get_compiled(hash(key), plan)

    base = {
        "wa": plan["wa_pk"],
        "wb": plan["wb_pk"],
        "br_c": np.ascontiguousarray(
            np.concatenate([plan["bias_c"], plan["resp_c"]], axis=1)),
    }

    x16 = inputs.astype(np.float16)
    in_maps = []
    for c in range(NCORES):
        m = dict(base)
        # node-major transpose: [256 nodes, 2048 batch]
        m["x"] = np.ascontiguousarray(x16[c * BC:(c + 1) * BC].T)
        in_maps.append(m)

    res = run_bass_kernel_spmd(nc, in_maps, core_ids=list(range(NCORES)))
    kernel.last_results = res
    colmap = np.asarray(plan["colmap"])
    out = np.concatenate(
        [res.results[c]["o"][colmap].T for c in range(NCORES)], axis=0)
    return np.ascontiguousarray(out.astype(np.float32))


kernel.last_results = None
